# revision 27
# baseline (speedup 1.0000x reference)
"""Single-head causal attention (B=8, T=2048, D=1024, H=128) on 8 TRN2 NeuronCores.

Sharding: one batch element per core (data-parallel over B).

Per-core algorithm, all matmuls bf16 (full PE rate at any width), fp32 PSUM:
  - host supplies x^T as [chunk, part, d, 512] bf16 and packed W [part, 3, d, h]
  - per 512-wide q-chunk: Q^T/K^T/V^T = W^T @ x^T (8 d-tile PSUM accumulate),
    DVE-cast to bf16 SBUF; V^T -> V[k,h] tiles via DMA-engine transpose
  - attention per chunk over k-tile PAIRS: S^T[k,q] for two k-tiles into one
    2-bank PSUM tile, ONE exp over [128,<=1024] on ACT (scale folded in),
    causal masks via gpsimd affine_select narrowed to the triangular window
    (also zeroes stale regions), O^T += V_j @ P^T accumulated in PSUM
  - row sums via a log-depth tree of wide bf16 DVE adds over the per-chunk
    P^T tile (few instructions; bf16 runs 2x on DVE), final fp32 combine
  - O^T (unnormalized, bf16) + row-sum partials (fp32) DMA'd out; host
    reduces partials and normalizes (cheap) - no on-chip reciprocal
  - schedule: chunks processed [1,2,3,0] so the cheapest chunk drains last;
    att(c) interleaved with later projections to fill PE bubbles; ACT exp
    table preloaded and PE HAM warmed with dummy matmuls during DMA fill;
    input DMA descriptors pushed from three different engine queues in
    parallel (a single queue serializes at ~0.7us per push).
"""
import numpy as np
import ml_dtypes

B, T, D, H = 8, 2048, 1024, 128
ND = D // 128      # 8 d-tiles
NTK = T // 128     # 16 k-tiles
NCH = T // 512     # 4 q-chunks
SCALE = float(H) ** -0.5

_CACHE = {}


def _build():
    import concourse.bass as bass  # noqa: F401
    from concourse import bacc
    import concourse.mybir as mybir
    import concourse.tile as tile

    f32 = mybir.dt.float32
    bf16 = mybir.dt.bfloat16

    nc = bacc.Bacc("TRN2", target_bir_lowering=False)
    xt_d = nc.dram_tensor("xt", (NCH, 128, ND, 512), bf16, kind="ExternalInput")
    w_d = nc.dram_tensor("w", (128, 3, ND, H), bf16, kind="ExternalInput")
    ot_d = nc.dram_tensor("ot", (H, T), bf16, kind="ExternalOutput")
    pc_d = nc.dram_tensor("pacc", (128, T), f32, kind="ExternalOutput")

    with tile.TileContext(nc) as tc:
        with (
            tc.tile_pool(name="sb", bufs=1) as sb,
            tc.tile_pool(name="ps", bufs=1, space="PSUM") as ps,
        ):
            xt = sb.tile([128, NCH, ND, 512], bf16, tag="xt")
            w = sb.tile([128, 3, ND, H], bf16, tag="w")
            qt = sb.tile([128, NCH, 512], bf16, tag="qt")
            kt = sb.tile([128, NTK, H], bf16, tag="kt")
            v = sb.tile([128, NTK, H], bf16, tag="v")
            wdum = sb.tile([128, 512], bf16, tag="wdum")
            warm = sb.tile([1, 16], f32, tag="warm")

            # ---- gpsimd: memsets first (unblock PE warm-up ASAP) ----
            nc.gpsimd.memset(wdum[:], 0.0)
            nc.gpsimd.memset(warm[:], 0.0)

            # ---- input DMA pushes spread across both HWDGE queues ----
            # weights first on each queue (they gate the projections), then
            # x chunk 0 pieces, then later chunks
            nc.sync.dma_start(w[:, 0, :, :], w_d[:, 0, :, :])
            nc.scalar.dma_start(w[:, 1, :, :], w_d[:, 1, :, :])
            nc.sync.dma_start(xt[:, 0, 2:4, :], xt_d[0, :, 2:4, :])
            nc.scalar.dma_start(xt[:, 0, 0:2, :], xt_d[0, :, 0:2, :])
            nc.sync.dma_start(xt[:, 0, 6:8, :], xt_d[0, :, 6:8, :])
            nc.scalar.dma_start(xt[:, 0, 4:6, :], xt_d[0, :, 4:6, :])
            nc.sync.dma_start(w[:, 2, :, :], w_d[:, 2, :, :])
            nc.scalar.dma_start(xt[:, 1, :, :], xt_d[1])
            nc.sync.dma_start(xt[:, 2, :, :], xt_d[2])
            nc.scalar.dma_start(xt[:, 3, :, :], xt_d[3])

            # ---- ACT exp-table preload ----
            nc.scalar.activation(warm[:], warm[:],
                                 mybir.ActivationFunctionType.Exp)

            # ---- PE warm-up (HAM ramp) on dummy weights during DMA fill ----
            warm_ps = ps.tile([128, 512], f32, tag="po", bufs=2, name="warmps")
            for i in range(14):
                nc.tensor.matmul(warm_ps[:], wdum[:, 0:128], wdum[:],
                                 start=True, stop=True)

            # ---- projection quanta for chunk c ----
            def proj_quanta(c):
                items = []

                def mk_proj(wi, dst):
                    acc = [None]

                    def mm(lo, hi):
                        def f():
                            if lo == 0:
                                acc[0] = ps.tile([128, 512], f32, tag="po",
                                                 bufs=2, name=f"acc{c}_{wi}")
                            for d in range(lo, hi):
                                nc.tensor.matmul(
                                    acc[0][:], w[:, wi, d, :], xt[:, c, d, :],
                                    start=(d == 0), stop=(d == ND - 1),
                                )
                        return f

                    def cast():
                        nc.vector.tensor_copy(dst, acc[0][:])
                    return [mm(0, 2), mm(2, 4), mm(4, 6), mm(6, 8), cast]

                vt = sb.tile([128, 512], bf16, tag="vt", bufs=2, name=f"vt{c}")
                items += mk_proj(0, qt[:, c, :])
                items += mk_proj(1, kt[:, 4 * c:4 * c + 4, :])
                items += mk_proj(2, vt[:])

                def transp():
                    for s in range(4):
                        nc.sync.dma_start_transpose(
                            v[:, 4 * c + s, :], vt[:, 128 * s:128 * (s + 1)])
                items.append(transp)
                return items

            # ---- one attention pair (logical pair q -> k-tiles 2q, 2q+1),
            #      stored in pt slot `s` (emission order) ----
            def scores_exp(c, q, s, pt):
                j0, j1 = 2 * q, 2 * q + 1
                d0 = j0 >= 4 * c
                d1 = j1 >= 4 * c
                lo0 = 128 * (j0 - 4 * c) if d0 else 0
                lo1 = 128 * (j1 - 4 * c) if d1 else 0

                sc = ps.tile([128, 1024], f32, tag="sc", bufs=2,
                             name=f"sc{c}_{q}")
                nc.tensor.matmul(sc[:, lo0:512], kt[:, j0, :],
                                 qt[:, c, lo0:512], start=True, stop=True)
                nc.tensor.matmul(sc[:, 512 + lo1:1024], kt[:, j1, :],
                                 qt[:, c, lo1:512], start=True, stop=True)

                # exp over the full pair region; stale/masked cols are zeroed
                # by the affine_select masks below (stale scores are bounded,
                # so exp cannot overflow)
                nc.scalar.activation(
                    pt[:, s, :, :], sc[:],
                    mybir.ActivationFunctionType.Exp, scale=SCALE)
                for hh, dg, lo in ((0, d0, lo0), (1, d1, lo1)):
                    if dg:
                        m = (2 * q + hh) - 4 * c
                        wid = lo + 128
                        nc.gpsimd.affine_select(
                            out=pt[:, s, hh, 0:wid], in_=pt[:, s, hh, 0:wid],
                            compare_op=mybir.AluOpType.is_ge, fill=0.0,
                            base=-128 * m, pattern=[[1, wid]],
                            channel_multiplier=-1,
                        )

            def pv_mm(c, q, s, pt, otp, first, last):
                j0, j1 = 2 * q, 2 * q + 1
                lo0 = 128 * (j0 - 4 * c) if j0 >= 4 * c else 0
                lo1 = 128 * (j1 - 4 * c) if j1 >= 4 * c else 0
                nc.tensor.matmul(otp[:, lo0:512], v[:, j0, :],
                                 pt[:, s, 0, lo0:512],
                                 start=first, stop=False)
                nc.tensor.matmul(otp[:, lo1:512], v[:, j1, :],
                                 pt[:, s, 1, lo1:512],
                                 start=False, stop=last)

            # ---- attention chunk c: returns list of thunks ----
            def att_items(c):
                npairs = 2 * (c + 1)
                state = {}
                items = []

                def start():
                    state['ot'] = ps.tile([128, 512], f32, tag="ot", bufs=2,
                                          name=f"ot{c}")
                    state['pt'] = sb.tile([128, 8, 2, 512], bf16, tag="pt",
                                          bufs=2, name=f"pt{c}")
                    state['pacc'] = sb.tile([128, 512], f32, tag="pacc",
                                            bufs=2, name=f"pacc{c}")
                    state['t1'] = sb.tile([128, 2, 512], bf16, tag="t1",
                                          bufs=2, name=f"t1{c}")
                    state['t2'] = sb.tile([128, 2, 512], bf16, tag="t2",
                                          bufs=2, name=f"t2{c}")
                items.append(start)

                # progressive row-sum accumulation: after pairs (u, u+1) are
                # masked, one wide bf16 add makes their per-pair half-sums
                # (quantum), a second folds the quantum into a running bf16
                # accumulator; only the final fp32 combine is serial at the
                # chunk end.
                sum_after = {i: [] for i in range(npairs)}

                def add(o, i0, i1):
                    with nc.allow_low_precision(reason="softmax denominator"):
                        nc.vector.tensor_add(o, i0, i1)

                def quantum(u, first, last):
                    pt, t1, t2 = state['pt'], state['t1'], state['t2']
                    dst = t2 if first else t1
                    add(dst[:, 0:2, :], pt[:, u:u + 2, 0, :],
                        pt[:, u:u + 2, 1, :])
                    if not first:
                        add(t2[:, 0:2, :], t2[:, 0:2, :], t1[:, 0:2, :])
                    if last:
                        add(state['pacc'][:], t2[:, 0, :], t2[:, 1, :])

                for u in range(0, npairs, 2):
                    sum_after[u + 1].append(
                        lambda u=u: quantum(u, u == 0, u + 2 >= npairs))

                porder = list(range(npairs))

                for s in range(npairs):
                    def pair_step(s=s):
                        scores_exp(c, porder[s], s, state['pt'])
                        if s > 0:
                            pv_mm(c, porder[s - 1], s - 1, state['pt'],
                                  state['ot'], s - 1 == 0, False)
                        for f in sum_after.get(s - 1, ()):
                            f()
                    items.append(pair_step)

                def last():
                    pv_mm(c, porder[npairs - 1], npairs - 1, state['pt'],
                          state['ot'], npairs == 1, True)
                    for f in sum_after[npairs - 1]:
                        f()
                    osb = sb.tile([128, 512], bf16, tag="osb", bufs=2,
                                  name=f"osb{c}")
                    # pacc push first on scalar (doesn't depend on osb), then
                    # evacuate O^T on the ACT engine (overlaps DVE adds)
                    nc.scalar.dma_start(pc_d[:, 512 * c:512 * (c + 1)],
                                        state['pacc'][:])
                    nc.scalar.copy(osb[:], state['ot'][:])
                    nc.sync.dma_start(ot_d[:, 512 * c:512 * (c + 1)], osb[:])
                items.append(last)
                return items

            # ---- schedule ----
            def run_interleaved(att, filler):
                att[0]()
                pairs = att[1:-1]
                nf, np_ = len(filler), len(pairs)
                fi = 0
                for i, pair in enumerate(pairs):
                    pair()
                    target = (i + 1) * nf // np_
                    while fi < target:
                        filler[fi]()
                        fi += 1
                att[-1]()

            for it in proj_quanta(0):
                it()
            for it in proj_quanta(1):
                it()
            run_interleaved(att_items(1), proj_quanta(2))
            run_interleaved(att_items(2), proj_quanta(3))
            # att(0) (2 pairs, all-diagonal) rides inside att(3)'s stream so
            # that only its tiny finisher (2 tree adds + DMA) is in the tail
            a3 = att_items(3)
            a0 = att_items(0)
            a3[0]()
            a0[0]()
            for i, pair in enumerate(a3[1:-1]):
                pair()
                if i == 2:
                    a0[1]()       # att0 scores/exp pair 0
                if i == 4:
                    a0[2]()       # att0 pair 1 (+ pv of pair 0)
                if i == 6:
                    a0[-1]()      # att0 finisher rides inside att3's stream
            a3[-1]()

    nc.compile()
    return nc


def kernel(x, W_Q, W_K, W_V):
    from concourse import bass_utils

    if "nc" not in _CACHE:
        _CACHE["nc"] = _build()
    nc = _CACHE["nc"]

    bf = ml_dtypes.bfloat16

    def warr(W):
        return np.asarray(W, np.float32).reshape(ND, 128, H).transpose(1, 0, 2)

    wpack = np.ascontiguousarray(
        np.stack([warr(W_Q), warr(W_K), warr(W_V)], axis=1)).astype(bf)
    x = np.asarray(x, np.float32)
    in_maps = []
    for b in range(B):
        # xt layout [chunk, part, d, 512]:  A[c,p,d,j] = x[b][512c+j, 128d+p]
        xa = np.ascontiguousarray(
            x[b].reshape(NCH, 512, ND, 128).transpose(0, 3, 2, 1)).astype(bf)
        in_maps.append({"xt": xa, "w": wpack})
    _CACHE["in_maps"] = in_maps
    res = bass_utils.run_bass_kernel_spmd(nc, in_maps, core_ids=list(range(B)))
    out = np.empty((B, T, H), np.float32)
    for b in range(B):
        ot = np.asarray(res.results[b]["ot"], dtype=np.float32)  # [H, T]
        denom = res.results[b]["pacc"].sum(axis=0)               # [T]
        out[b] = (ot / denom[None, :]).T
    return out


# revision 28
# speedup vs baseline: 1.0193x; 1.0193x over previous
"""Single-head causal attention (B=8, T=2048, D=1024, H=128) on 8 TRN2 NeuronCores.

Sharding: one batch element per core (data-parallel over B).

Per-core algorithm, all matmuls bf16 (full PE rate at any width), fp32 PSUM:
  - host supplies x^T as [chunk, part, d, 512] bf16 and packed W [part, 3, d, h]
  - per 512-wide q-chunk: Q^T/K^T/V^T = W^T @ x^T (8 d-tile PSUM accumulate),
    DVE-cast to bf16 SBUF; V^T -> V[k,h] tiles via DMA-engine transpose
  - attention per chunk over k-tile PAIRS: S^T[k,q] for two k-tiles into one
    2-bank PSUM tile, ONE exp over [128,<=1024] on ACT (scale folded in),
    causal masks via gpsimd affine_select narrowed to the triangular window
    (also zeroes stale regions), O^T += V_j @ P^T accumulated in PSUM
  - row sums via a log-depth tree of wide bf16 DVE adds over the per-chunk
    P^T tile (few instructions; bf16 runs 2x on DVE), final fp32 combine
  - O^T (unnormalized, bf16) + row-sum partials (fp32) DMA'd out; host
    reduces partials and normalizes (cheap) - no on-chip reciprocal
  - schedule: chunks processed [1,2,3,0] so the cheapest chunk drains last;
    att(c) interleaved with later projections to fill PE bubbles; ACT exp
    table preloaded and PE HAM warmed with dummy matmuls during DMA fill;
    input DMA descriptors pushed from three different engine queues in
    parallel (a single queue serializes at ~0.7us per push).
"""
import numpy as np
import ml_dtypes

B, T, D, H = 8, 2048, 1024, 128
ND = D // 128      # 8 d-tiles
NTK = T // 128     # 16 k-tiles
NCH = T // 512     # 4 q-chunks
SCALE = float(H) ** -0.5

_CACHE = {}


def _build():
    import concourse.bass as bass  # noqa: F401
    from concourse import bacc
    import concourse.mybir as mybir
    import concourse.tile as tile

    f32 = mybir.dt.float32
    bf16 = mybir.dt.bfloat16

    nc = bacc.Bacc("TRN2", target_bir_lowering=False)
    xt_d = nc.dram_tensor("xt", (NCH, 128, ND, 512), bf16, kind="ExternalInput")
    w_d = nc.dram_tensor("w", (128, 3, ND, H), bf16, kind="ExternalInput")
    ot_d = nc.dram_tensor("ot", (H, T), bf16, kind="ExternalOutput")
    pc_d = nc.dram_tensor("pacc", (128, T), f32, kind="ExternalOutput")

    with tile.TileContext(nc) as tc:
        with (
            tc.tile_pool(name="sb", bufs=1) as sb,
            tc.tile_pool(name="ps", bufs=1, space="PSUM") as ps,
        ):
            xt = sb.tile([128, NCH, ND, 512], bf16, tag="xt")
            w = sb.tile([128, 3, ND, H], bf16, tag="w")
            qt = sb.tile([128, NCH, 512], bf16, tag="qt")
            kt = sb.tile([128, NTK, H], bf16, tag="kt")
            v = sb.tile([128, NTK, H], bf16, tag="v")
            wdum = sb.tile([128, 512], bf16, tag="wdum")
            warm = sb.tile([1, 16], f32, tag="warm")

            # ---- gpsimd: memsets first (unblock PE warm-up ASAP) ----
            nc.gpsimd.memset(wdum[:], 0.0)
            nc.gpsimd.memset(warm[:], 0.0)

            # ---- input DMA pushes spread across both HWDGE queues ----
            # weights first on each queue (they gate the projections), then
            # x chunk 0 pieces, then later chunks
            nc.sync.dma_start(w[:, 0, :, :], w_d[:, 0, :, :])
            nc.scalar.dma_start(w[:, 1, :, :], w_d[:, 1, :, :])
            nc.sync.dma_start(xt[:, 0, 2:4, :], xt_d[0, :, 2:4, :])
            nc.scalar.dma_start(xt[:, 0, 0:2, :], xt_d[0, :, 0:2, :])
            nc.sync.dma_start(xt[:, 0, 6:8, :], xt_d[0, :, 6:8, :])
            nc.scalar.dma_start(xt[:, 0, 4:6, :], xt_d[0, :, 4:6, :])
            nc.sync.dma_start(w[:, 2, :, :], w_d[:, 2, :, :])
            nc.scalar.dma_start(xt[:, 1, :, :], xt_d[1])
            nc.sync.dma_start(xt[:, 2, :, :], xt_d[2])
            # x chunk 3 rides the gpsimd software-DGE path: a third parallel
            # DMA stream, and gpsimd is idle until the first masks (~20us)
            nc.gpsimd.dma_start(xt[:, 3, :, :], xt_d[3])

            # ---- ACT exp-table preload ----
            nc.scalar.activation(warm[:], warm[:],
                                 mybir.ActivationFunctionType.Exp)

            # ---- PE warm-up (HAM ramp) on dummy weights during DMA fill ----
            warm_ps = ps.tile([128, 512], f32, tag="po", bufs=2, name="warmps")
            for i in range(14):
                nc.tensor.matmul(warm_ps[:], wdum[:, 0:128], wdum[:],
                                 start=True, stop=True)

            # ---- projection quanta for chunk c ----
            def proj_quanta(c):
                items = []

                def mk_proj(wi, dst):
                    acc = [None]

                    def mm(lo, hi):
                        def f():
                            if lo == 0:
                                acc[0] = ps.tile([128, 512], f32, tag="po",
                                                 bufs=2, name=f"acc{c}_{wi}")
                            for d in range(lo, hi):
                                nc.tensor.matmul(
                                    acc[0][:], w[:, wi, d, :], xt[:, c, d, :],
                                    start=(d == 0), stop=(d == ND - 1),
                                )
                        return f

                    def cast():
                        nc.vector.tensor_copy(dst, acc[0][:])
                    return [mm(0, 2), mm(2, 4), mm(4, 6), mm(6, 8), cast]

                vt = sb.tile([128, 512], bf16, tag="vt", bufs=2, name=f"vt{c}")
                items += mk_proj(0, qt[:, c, :])
                items += mk_proj(1, kt[:, 4 * c:4 * c + 4, :])
                items += mk_proj(2, vt[:])

                def transp():
                    for s in range(4):
                        nc.sync.dma_start_transpose(
                            v[:, 4 * c + s, :], vt[:, 128 * s:128 * (s + 1)])
                items.append(transp)
                return items

            # ---- one attention pair (logical pair q -> k-tiles 2q, 2q+1),
            #      stored in pt slot `s` (emission order) ----
            def scores_exp(c, q, s, pt):
                j0, j1 = 2 * q, 2 * q + 1
                d0 = j0 >= 4 * c
                d1 = j1 >= 4 * c
                lo0 = 128 * (j0 - 4 * c) if d0 else 0
                lo1 = 128 * (j1 - 4 * c) if d1 else 0

                sc = ps.tile([128, 1024], f32, tag="sc", bufs=2,
                             name=f"sc{c}_{q}")
                nc.tensor.matmul(sc[:, lo0:512], kt[:, j0, :],
                                 qt[:, c, lo0:512], start=True, stop=True)
                nc.tensor.matmul(sc[:, 512 + lo1:1024], kt[:, j1, :],
                                 qt[:, c, lo1:512], start=True, stop=True)

                # exp over the full pair region; stale/masked cols are zeroed
                # by the affine_select masks below (stale scores are bounded,
                # so exp cannot overflow)
                nc.scalar.activation(
                    pt[:, s, :, :], sc[:],
                    mybir.ActivationFunctionType.Exp, scale=SCALE)
                for hh, dg, lo in ((0, d0, lo0), (1, d1, lo1)):
                    if dg:
                        m = (2 * q + hh) - 4 * c
                        wid = lo + 128
                        nc.gpsimd.affine_select(
                            out=pt[:, s, hh, 0:wid], in_=pt[:, s, hh, 0:wid],
                            compare_op=mybir.AluOpType.is_ge, fill=0.0,
                            base=-128 * m, pattern=[[1, wid]],
                            channel_multiplier=-1,
                        )

            def pv_mm(c, q, s, pt, otp, first, last):
                j0, j1 = 2 * q, 2 * q + 1
                lo0 = 128 * (j0 - 4 * c) if j0 >= 4 * c else 0
                lo1 = 128 * (j1 - 4 * c) if j1 >= 4 * c else 0
                nc.tensor.matmul(otp[:, lo0:512], v[:, j0, :],
                                 pt[:, s, 0, lo0:512],
                                 start=first, stop=False)
                nc.tensor.matmul(otp[:, lo1:512], v[:, j1, :],
                                 pt[:, s, 1, lo1:512],
                                 start=False, stop=last)

            # ---- attention chunk c: returns list of thunks ----
            def att_items(c):
                npairs = 2 * (c + 1)
                state = {}
                items = []

                def start():
                    state['ot'] = ps.tile([128, 512], f32, tag="ot", bufs=2,
                                          name=f"ot{c}")
                    state['pt'] = sb.tile([128, 8, 2, 512], bf16, tag="pt",
                                          bufs=2, name=f"pt{c}")
                    state['pacc'] = sb.tile([128, 512], f32, tag="pacc",
                                            bufs=2, name=f"pacc{c}")
                    state['t1'] = sb.tile([128, 2, 512], bf16, tag="t1",
                                          bufs=2, name=f"t1{c}")
                    state['t2'] = sb.tile([128, 2, 512], bf16, tag="t2",
                                          bufs=2, name=f"t2{c}")
                items.append(start)

                # progressive row-sum accumulation: after pairs (u, u+1) are
                # masked, one wide bf16 add makes their per-pair half-sums
                # (quantum), a second folds the quantum into a running bf16
                # accumulator; only the final fp32 combine is serial at the
                # chunk end.
                sum_after = {i: [] for i in range(npairs)}

                def add(o, i0, i1):
                    with nc.allow_low_precision(reason="softmax denominator"):
                        nc.vector.tensor_add(o, i0, i1)

                def quantum(u, first, last):
                    pt, t1, t2 = state['pt'], state['t1'], state['t2']
                    dst = t2 if first else t1
                    add(dst[:, 0:2, :], pt[:, u:u + 2, 0, :],
                        pt[:, u:u + 2, 1, :])
                    if not first:
                        add(t2[:, 0:2, :], t2[:, 0:2, :], t1[:, 0:2, :])
                    if last:
                        add(state['pacc'][:], t2[:, 0, :], t2[:, 1, :])

                for u in range(0, npairs, 2):
                    sum_after[u + 1].append(
                        lambda u=u: quantum(u, u == 0, u + 2 >= npairs))

                porder = list(range(npairs))

                for s in range(npairs):
                    def pair_step(s=s):
                        scores_exp(c, porder[s], s, state['pt'])
                        if s > 0:
                            pv_mm(c, porder[s - 1], s - 1, state['pt'],
                                  state['ot'], s - 1 == 0, False)
                        for f in sum_after.get(s - 1, ()):
                            f()
                    items.append(pair_step)

                def last():
                    pv_mm(c, porder[npairs - 1], npairs - 1, state['pt'],
                          state['ot'], npairs == 1, True)
                    for f in sum_after[npairs - 1]:
                        f()
                    osb = sb.tile([128, 512], bf16, tag="osb", bufs=2,
                                  name=f"osb{c}")
                    # pacc push first on scalar (doesn't depend on osb), then
                    # evacuate O^T on the ACT engine (overlaps DVE adds)
                    nc.scalar.dma_start(pc_d[:, 512 * c:512 * (c + 1)],
                                        state['pacc'][:])
                    nc.scalar.copy(osb[:], state['ot'][:])
                    nc.sync.dma_start(ot_d[:, 512 * c:512 * (c + 1)], osb[:])
                items.append(last)
                return items

            # ---- schedule ----
            def run_interleaved(att, filler):
                att[0]()
                pairs = att[1:-1]
                nf, np_ = len(filler), len(pairs)
                fi = 0
                for i, pair in enumerate(pairs):
                    pair()
                    target = (i + 1) * nf // np_
                    while fi < target:
                        filler[fi]()
                        fi += 1
                att[-1]()

            for it in proj_quanta(0):
                it()
            for it in proj_quanta(1):
                it()
            run_interleaved(att_items(1), proj_quanta(2))
            run_interleaved(att_items(2), proj_quanta(3))
            # att(0) (2 pairs, all-diagonal) rides inside att(3)'s stream so
            # that only its tiny finisher (2 tree adds + DMA) is in the tail
            a3 = att_items(3)
            a0 = att_items(0)
            a3[0]()
            a0[0]()
            for i, pair in enumerate(a3[1:-1]):
                pair()
                if i == 2:
                    a0[1]()       # att0 scores/exp pair 0
                if i == 4:
                    a0[2]()       # att0 pair 1 (+ pv of pair 0)
                if i == 6:
                    a0[-1]()      # att0 finisher rides inside att3's stream
            a3[-1]()

    nc.compile()
    return nc


def kernel(x, W_Q, W_K, W_V):
    from concourse import bass_utils

    if "nc" not in _CACHE:
        _CACHE["nc"] = _build()
    nc = _CACHE["nc"]

    bf = ml_dtypes.bfloat16

    def warr(W):
        return np.asarray(W, np.float32).reshape(ND, 128, H).transpose(1, 0, 2)

    wpack = np.ascontiguousarray(
        np.stack([warr(W_Q), warr(W_K), warr(W_V)], axis=1)).astype(bf)
    x = np.asarray(x, np.float32)
    in_maps = []
    for b in range(B):
        # xt layout [chunk, part, d, 512]:  A[c,p,d,j] = x[b][512c+j, 128d+p]
        xa = np.ascontiguousarray(
            x[b].reshape(NCH, 512, ND, 128).transpose(0, 3, 2, 1)).astype(bf)
        in_maps.append({"xt": xa, "w": wpack})
    _CACHE["in_maps"] = in_maps
    res = bass_utils.run_bass_kernel_spmd(nc, in_maps, core_ids=list(range(B)))
    out = np.empty((B, T, H), np.float32)
    for b in range(B):
        ot = np.asarray(res.results[b]["ot"], dtype=np.float32)  # [H, T]
        denom = res.results[b]["pacc"].sum(axis=0)               # [T]
        out[b] = (ot / denom[None, :]).T
    return out


# revision 30
# speedup vs baseline: 1.0535x; 1.0335x over previous
"""Single-head causal attention (B=8, T=2048, D=1024, H=128) on 8 TRN2 NeuronCores.

Sharding: one batch element per core (data-parallel over B).

Per-core algorithm, all matmuls bf16 (full PE rate at any width), fp32 PSUM:
  - host supplies x^T as [chunk, part, d, 512] bf16 and packed W [part, 3, d, h]
  - per 512-wide q-chunk: Q^T/K^T/V^T = W^T @ x^T (8 d-tile PSUM accumulate),
    DVE-cast to bf16 SBUF; V^T -> V[k,h] tiles via DMA-engine transpose
  - attention per chunk over k-tile PAIRS: S^T[k,q] for two k-tiles into one
    2-bank PSUM tile, ONE exp over [128,<=1024] on ACT (scale folded in),
    causal masks via gpsimd affine_select narrowed to the triangular window
    (also zeroes stale regions), O^T += V_j @ P^T accumulated in PSUM
  - row sums via a log-depth tree of wide bf16 DVE adds over the per-chunk
    P^T tile (few instructions; bf16 runs 2x on DVE), final fp32 combine
  - O^T (unnormalized, bf16) + row-sum partials (fp32) DMA'd out; host
    reduces partials and normalizes (cheap) - no on-chip reciprocal
  - schedule: chunks processed [1,2,3,0] so the cheapest chunk drains last;
    att(c) interleaved with later projections to fill PE bubbles; ACT exp
    table preloaded and PE HAM warmed with dummy matmuls during DMA fill;
    input DMA descriptors pushed from three different engine queues in
    parallel (a single queue serializes at ~0.7us per push).
"""
import numpy as np
import ml_dtypes

B, T, D, H = 8, 2048, 1024, 128
ND = D // 128      # 8 d-tiles
NTK = T // 128     # 16 k-tiles
NCH = T // 512     # 4 q-chunks
SCALE = float(H) ** -0.5

_CACHE = {}


def _build():
    import concourse.bass as bass  # noqa: F401
    from concourse import bacc
    import concourse.mybir as mybir
    import concourse.tile as tile

    f32 = mybir.dt.float32
    bf16 = mybir.dt.bfloat16

    nc = bacc.Bacc("TRN2", target_bir_lowering=False)
    xt_d = nc.dram_tensor("xt", (NCH, 128, ND, 512), bf16, kind="ExternalInput")
    w_d = nc.dram_tensor("w", (128, 3, ND, H), bf16, kind="ExternalInput")
    ot_d = nc.dram_tensor("ot", (H, T), bf16, kind="ExternalOutput")
    pc_d = nc.dram_tensor("pacc", (128, T), f32, kind="ExternalOutput")

    with tile.TileContext(nc) as tc:
        with (
            tc.tile_pool(name="sb", bufs=1) as sb,
            tc.tile_pool(name="ps", bufs=1, space="PSUM") as ps,
        ):
            xt = sb.tile([128, NCH, ND, 512], bf16, tag="xt")
            w = sb.tile([128, 3, ND, H], bf16, tag="w")
            qt = sb.tile([128, NCH, 512], bf16, tag="qt")
            kt = sb.tile([128, NTK, H], bf16, tag="kt")
            v = sb.tile([128, NTK, H], bf16, tag="v")
            wdum = sb.tile([128, 512], bf16, tag="wdum")
            warm = sb.tile([1, 16], f32, tag="warm")

            # ---- gpsimd: memsets first (unblock PE warm-up ASAP) ----
            nc.gpsimd.memset(wdum[:], 0.0)
            nc.gpsimd.memset(warm[:], 0.0)

            # ---- input DMA pushes spread across both HWDGE queues ----
            # weights first on each queue (they gate the projections), then
            # x chunk 0 pieces, then later chunks
            nc.sync.dma_start(w[:, 0, :, :], w_d[:, 0, :, :])
            nc.scalar.dma_start(w[:, 1, :, :], w_d[:, 1, :, :])
            nc.sync.dma_start(xt[:, 0, 2:4, :], xt_d[0, :, 2:4, :])
            nc.scalar.dma_start(xt[:, 0, 0:2, :], xt_d[0, :, 0:2, :])
            nc.sync.dma_start(xt[:, 0, 6:8, :], xt_d[0, :, 6:8, :])
            nc.scalar.dma_start(xt[:, 0, 4:6, :], xt_d[0, :, 4:6, :])
            nc.sync.dma_start(w[:, 2, :, :], w_d[:, 2, :, :])
            nc.scalar.dma_start(xt[:, 1, :, :], xt_d[1])
            nc.sync.dma_start(xt[:, 2, :, :], xt_d[2])
            # x chunk 3's push is emitted later (inside att(1)'s stream) so
            # its transfer doesn't steal DMA bandwidth from chunk 0

            # ---- ACT exp-table preload ----
            nc.scalar.activation(warm[:], warm[:],
                                 mybir.ActivationFunctionType.Exp)

            # ---- PE warm-up (HAM ramp) on dummy weights during DMA fill ----
            warm_ps = ps.tile([128, 512], f32, tag="po", bufs=2, name="warmps")
            for i in range(14):
                nc.tensor.matmul(warm_ps[:], wdum[:, 0:128], wdum[:],
                                 start=True, stop=True)

            # ---- projection quanta for chunk c ----
            def proj_quanta(c):
                items = []

                def mk_proj(wi, dst):
                    acc = [None]

                    def mm(lo, hi):
                        def f():
                            if lo == 0:
                                acc[0] = ps.tile([128, 512], f32, tag="po",
                                                 bufs=2, name=f"acc{c}_{wi}")
                            for d in range(lo, hi):
                                nc.tensor.matmul(
                                    acc[0][:], w[:, wi, d, :], xt[:, c, d, :],
                                    start=(d == 0), stop=(d == ND - 1),
                                )
                        return f

                    def cast():
                        nc.vector.tensor_copy(dst, acc[0][:])
                    return [mm(0, 2), mm(2, 4), mm(4, 6), mm(6, 8), cast]

                vt = sb.tile([128, 512], bf16, tag="vt", bufs=2, name=f"vt{c}")
                items += mk_proj(0, qt[:, c, :])
                items += mk_proj(1, kt[:, 4 * c:4 * c + 4, :])
                items += mk_proj(2, vt[:])

                def transp():
                    for s in range(4):
                        nc.sync.dma_start_transpose(
                            v[:, 4 * c + s, :], vt[:, 128 * s:128 * (s + 1)])
                items.append(transp)
                return items

            # ---- one attention pair (logical pair q -> k-tiles 2q, 2q+1),
            #      stored in pt slot `s` (emission order) ----
            def scores_exp(c, q, s, pt):
                j0, j1 = 2 * q, 2 * q + 1
                d0 = j0 >= 4 * c
                d1 = j1 >= 4 * c
                lo0 = 128 * (j0 - 4 * c) if d0 else 0
                lo1 = 128 * (j1 - 4 * c) if d1 else 0

                sc = ps.tile([128, 1024], f32, tag="sc", bufs=2,
                             name=f"sc{c}_{q}")
                nc.tensor.matmul(sc[:, lo0:512], kt[:, j0, :],
                                 qt[:, c, lo0:512], start=True, stop=True)
                nc.tensor.matmul(sc[:, 512 + lo1:1024], kt[:, j1, :],
                                 qt[:, c, lo1:512], start=True, stop=True)

                # exp over the full pair region; stale/masked cols are zeroed
                # by the affine_select masks below (stale scores are bounded,
                # so exp cannot overflow)
                nc.scalar.activation(
                    pt[:, s, :, :], sc[:],
                    mybir.ActivationFunctionType.Exp, scale=SCALE)
                for hh, dg, lo in ((0, d0, lo0), (1, d1, lo1)):
                    if dg:
                        m = (2 * q + hh) - 4 * c
                        wid = lo + 128
                        nc.gpsimd.affine_select(
                            out=pt[:, s, hh, 0:wid], in_=pt[:, s, hh, 0:wid],
                            compare_op=mybir.AluOpType.is_ge, fill=0.0,
                            base=-128 * m, pattern=[[1, wid]],
                            channel_multiplier=-1,
                        )

            def pv_mm(c, q, s, pt, otp, first, last):
                j0, j1 = 2 * q, 2 * q + 1
                lo0 = 128 * (j0 - 4 * c) if j0 >= 4 * c else 0
                lo1 = 128 * (j1 - 4 * c) if j1 >= 4 * c else 0
                nc.tensor.matmul(otp[:, lo0:512], v[:, j0, :],
                                 pt[:, s, 0, lo0:512],
                                 start=first, stop=False)
                nc.tensor.matmul(otp[:, lo1:512], v[:, j1, :],
                                 pt[:, s, 1, lo1:512],
                                 start=False, stop=last)

            # ---- attention chunk c: returns list of thunks ----
            def att_items(c):
                npairs = 2 * (c + 1)
                state = {}
                items = []

                def start():
                    state['ot'] = ps.tile([128, 512], f32, tag="ot", bufs=2,
                                          name=f"ot{c}")
                    state['pt'] = sb.tile([128, 8, 2, 512], bf16, tag="pt",
                                          bufs=2, name=f"pt{c}")
                    state['pacc'] = sb.tile([128, 512], f32, tag="pacc",
                                            bufs=2, name=f"pacc{c}")
                    state['t1'] = sb.tile([128, 2, 512], bf16, tag="t1",
                                          bufs=2, name=f"t1{c}")
                    state['t2'] = sb.tile([128, 2, 512], bf16, tag="t2",
                                          bufs=2, name=f"t2{c}")
                items.append(start)

                # progressive row-sum accumulation: after pairs (u, u+1) are
                # masked, one wide bf16 add makes their per-pair half-sums
                # (quantum), a second folds the quantum into a running bf16
                # accumulator; only the final fp32 combine is serial at the
                # chunk end.
                sum_after = {i: [] for i in range(npairs)}

                def add(o, i0, i1):
                    with nc.allow_low_precision(reason="softmax denominator"):
                        nc.vector.tensor_add(o, i0, i1)

                def quantum(u, first, last):
                    pt, t1, t2 = state['pt'], state['t1'], state['t2']
                    dst = t2 if first else t1
                    add(dst[:, 0:2, :], pt[:, u:u + 2, 0, :],
                        pt[:, u:u + 2, 1, :])
                    if not first:
                        add(t2[:, 0:2, :], t2[:, 0:2, :], t1[:, 0:2, :])
                    if last:
                        add(state['pacc'][:], t2[:, 0, :], t2[:, 1, :])

                for u in range(0, npairs, 2):
                    sum_after[u + 1].append(
                        lambda u=u: quantum(u, u == 0, u + 2 >= npairs))

                porder = list(range(npairs))

                for s in range(npairs):
                    def pair_step(s=s):
                        scores_exp(c, porder[s], s, state['pt'])
                        if s > 0:
                            pv_mm(c, porder[s - 1], s - 1, state['pt'],
                                  state['ot'], s - 1 == 0, False)
                        for f in sum_after.get(s - 1, ()):
                            f()
                    items.append(pair_step)

                def last():
                    pv_mm(c, porder[npairs - 1], npairs - 1, state['pt'],
                          state['ot'], npairs == 1, True)
                    for f in sum_after[npairs - 1]:
                        f()
                    osb = sb.tile([128, 512], bf16, tag="osb", bufs=2,
                                  name=f"osb{c}")
                    # pacc push first on scalar (doesn't depend on osb), then
                    # evacuate O^T on the ACT engine (overlaps DVE adds)
                    nc.scalar.dma_start(pc_d[:, 512 * c:512 * (c + 1)],
                                        state['pacc'][:])
                    nc.scalar.copy(osb[:], state['ot'][:])
                    nc.sync.dma_start(ot_d[:, 512 * c:512 * (c + 1)], osb[:])
                items.append(last)
                return items

            # ---- schedule ----
            def run_interleaved(att, filler):
                att[0]()
                pairs = att[1:-1]
                nf, np_ = len(filler), len(pairs)
                fi = 0
                for i, pair in enumerate(pairs):
                    pair()
                    target = (i + 1) * nf // np_
                    while fi < target:
                        filler[fi]()
                        fi += 1
                att[-1]()

            for it in proj_quanta(0):
                it()
            for it in proj_quanta(1):
                it()
            def push_x3():
                nc.scalar.dma_start(xt[:, 3, :, :], xt_d[3])
            run_interleaved(att_items(1), [push_x3] + proj_quanta(2))
            run_interleaved(att_items(2), proj_quanta(3))
            # att(0) (2 pairs, all-diagonal) rides inside att(3)'s stream so
            # that only its tiny finisher (2 tree adds + DMA) is in the tail
            a3 = att_items(3)
            a0 = att_items(0)
            a3[0]()
            a0[0]()
            for i, pair in enumerate(a3[1:-1]):
                pair()
                if i == 2:
                    a0[1]()       # att0 scores/exp pair 0
                if i == 4:
                    a0[2]()       # att0 pair 1 (+ pv of pair 0)
                if i == 6:
                    a0[-1]()      # att0 finisher rides inside att3's stream
            a3[-1]()

    nc.compile()
    return nc


def kernel(x, W_Q, W_K, W_V):
    from concourse import bass_utils

    if "nc" not in _CACHE:
        _CACHE["nc"] = _build()
    nc = _CACHE["nc"]

    bf = ml_dtypes.bfloat16

    def warr(W):
        return np.asarray(W, np.float32).reshape(ND, 128, H).transpose(1, 0, 2)

    wpack = np.ascontiguousarray(
        np.stack([warr(W_Q), warr(W_K), warr(W_V)], axis=1)).astype(bf)
    x = np.asarray(x, np.float32)
    in_maps = []
    for b in range(B):
        # xt layout [chunk, part, d, 512]:  A[c,p,d,j] = x[b][512c+j, 128d+p]
        xa = np.ascontiguousarray(
            x[b].reshape(NCH, 512, ND, 128).transpose(0, 3, 2, 1)).astype(bf)
        in_maps.append({"xt": xa, "w": wpack})
    _CACHE["in_maps"] = in_maps
    res = bass_utils.run_bass_kernel_spmd(nc, in_maps, core_ids=list(range(B)))
    out = np.empty((B, T, H), np.float32)
    for b in range(B):
        ot = np.asarray(res.results[b]["ot"], dtype=np.float32)  # [H, T]
        denom = res.results[b]["pacc"].sum(axis=0)               # [T]
        out[b] = (ot / denom[None, :]).T
    return out
